# revision 1
# baseline (speedup 1.0000x reference)
"""Trainium2 Bass kernel for nn_CESAR_24309514895978 (ragged_sequence).

Math (per batch b):
  m0 = (attention_masks==1)&(token_type_ids==0); m1 = (attention_masks==1)&(token_type_ids==1)
  score[i,j] = |emb_n[i] . emb_n[j]|   (L2-normalized embeddings)
  logits[i,j] = (emb@Wq.T+bq)[i] . (emb@Wk.T+bk)[j]
  cs[b] = sum_{valid ij} softmax_flat(logits | pair_mask)[i,j] * score[i,j]

Constant folding (host, once): the projections only enter through
  logits = embaug @ A_aug @ embaug.T,  embaug = [emb, 1],
  A_aug = [[Wq.T@Wk, Wq.T@bk], [bq.T@Wk, bq.bk]]   ((D+1)x(D+1))
so the per-batch device work is two chained matmuls instead of three.

Device, per batch (data-parallel: 2 batches per core x 8 cores, fp32r matmuls):
  - rsq[j] = sum_d emb[j,d]^2 (DVE squares+adds, one ones-column matmul);
    r = 1/sqrt (ACT Sqrt + DVE reciprocal); W2 = r row via gpsimd broadcast
  - P = A_aug @ embaug.T   (stage 1, 8 PSUM banks, db-outer accumulation;
    the u-row rides the PSUM->SBUF copy bias, c0 rides the prow copy bias)
  - L = embaug.T.T @ P + one K=3 matmul adding the -1e30 ragged-pair masks
    and the rank-1 prow term (rows: m0neg/ones/ones x ones/m1neg/prow)
  - M = masked max (DVE reduces + gpsimd partition_all_reduce)
  - E = exp(L - M) on ACT with accum_out -> Z partial sums
  - W partials = sum_j E * |G| * r_j  (G = gram matmul; |.| on ACT; stt fused)
Host: r_i scaling + final sums + W/Z division (tiny) + input layout/rounding.
"""
import numpy as np

import concourse.bass_isa as bass_isa
import concourse.tile as tile
from concourse import bacc, mybir
from concourse.bass_utils import run_bass_kernel_spmd

B, S, D = 16, 512, 1024
NCORES = 8
BPC = B // NCORES          # batches per core
NCH = D // 128             # 8 contraction chunks
NIC = S // 128             # 4 i-chunks
DA = D + 1                 # augmented dim
NEG = np.float32(-1e30)

F32 = mybir.dt.float32
F32R = mybir.dt.float32r
AFT = mybir.ActivationFunctionType
ALU = mybir.AluOpType
AX = mybir.AxisListType

PROFILE = False            # set True (e.g. from test.py) to capture NTFF profile
LAST_RESULTS = None        # BassKernelResults of the last run (for test.py)

_built = None


def _to_fp32r(x: np.ndarray) -> np.ndarray:
    """Round fp32 -> fp32r encoding (RNE to 11 explicit mantissa bits)."""
    u = np.ascontiguousarray(x, dtype=np.float32).view(np.uint32).astype(np.uint64)
    u = (u + 0x7FF + ((u >> 12) & 1)) & np.uint64(0xFFFFF000)
    return u.astype(np.uint32).view(np.float32)


def _build():
    global _built
    if _built is not None:
        return _built

    nc = bacc.Bacc("TRN2", target_bir_lowering=False, debug=False)

    embT_d = nc.dram_tensor("embT", [BPC, NCH, 128, S], F32R, kind="ExternalInput").ap()
    # AT[db, da] = A_aug[da, db]; rows 0..1023 in 8 chunks + row 1024 separate
    at_d = nc.dram_tensor("at", [DA, DA], F32R, kind="ExternalInput").ap()
    lrows_d = nc.dram_tensor("lrows", [BPC, 3, S], F32R, kind="ExternalInput").ap()
    rrows_d = nc.dram_tensor("rrows", [BPC, 2, S], F32R, kind="ExternalInput").ap()
    ucol_d = nc.dram_tensor("ucol", [128, NCH], F32, kind="ExternalInput").ap()
    c0_d = nc.dram_tensor("c0", [1, 1], F32, kind="ExternalInput").ap()
    ones_d = nc.dram_tensor("ones", [128, 1], F32R, kind="ExternalInput").ap()
    onesrow_d = nc.dram_tensor("onesrow", [1, S], F32R, kind="ExternalInput").ap()

    zw_d = nc.dram_tensor("zw", [BPC, 2, 128, NIC], F32, kind="ExternalOutput").ap()
    rout_d = nc.dram_tensor("rout", [BPC, S], F32, kind="ExternalOutput").ap()

    with tile.TileContext(nc) as tc:
        with (
            tc.tile_pool(name="apool", bufs=9) as apool,
            tc.tile_pool(name="spool", bufs=1) as spool,
            tc.tile_pool(name="epool", bufs=16) as epool,
            tc.tile_pool(name="sqpool", bufs=3) as sqpool,
            tc.tile_pool(name="paugpool", bufs=18) as paugpool,
            tc.tile_pool(name="w2pool", bufs=2) as w2pool,
            tc.tile_pool(name="gapool", bufs=2) as gapool,
            tc.tile_pool(name="gwpool", bufs=4) as gwpool,
            tc.tile_pool(name="Epool", bufs=2) as Epool,
            tc.tile_pool(name="scrpool", bufs=1) as scrpool,
            tc.tile_pool(name="tiny", bufs=2) as tiny,
            tc.tile_pool(name="lrpool", bufs=2) as lrpool,
            tc.tile_pool(name="ps", bufs=8, space="PSUM") as ps,
        ):
            # ---- first chunk pair goes absolutely first (PE start gate),
            # then the tiny loads, then the remaining big chunks interleaved.
            emb_all = [[None] * NCH for _ in range(BPC)]
            at_t = []
            t = apool.tile([128, DA], F32R, tag="a", name="at_0")
            nc.sync.dma_start(out=t[:], in_=at_d[0:128, :])
            at_t.append(t)
            t = epool.tile([128, S], F32R, tag="emb", name="emb0_0")
            nc.sync.dma_start(out=t[:], in_=embT_d[0, 0])
            emb_all[0][0] = t

            ones_col = spool.tile([128, 1], F32R, tag="ones_col")
            nc.sync.dma_start(out=ones_col[:], in_=ones_d)
            onesrow_t = spool.tile([1, S], F32R, tag="onesrow")
            nc.sync.dma_start(out=onesrow_t[:], in_=onesrow_d)
            ucol_t = spool.tile([128, NCH], F32, tag="ucol")
            nc.sync.dma_start(out=ucol_t[:], in_=ucol_d)
            c0_t = spool.tile([1, 1], F32, tag="c0")
            nc.sync.dma_start(out=c0_t[:], in_=c0_d)
            lr_all = []
            for b in range(BPC):
                lr_t = lrpool.tile([3, S], F32R, tag="lr", name=f"lr{b}")
                nc.sync.dma_start(out=lr_t[:], in_=lrows_d[b])
                lr_all.append(lr_t)

            for c in range(1, NCH):
                t = epool.tile([128, S], F32R, tag="emb", name=f"emb0_{c}")
                nc.sync.dma_start(out=t[:], in_=embT_d[0, c])
                emb_all[0][c] = t
                t = apool.tile([128, DA], F32R, tag="a", name=f"at_{c}")
                nc.sync.dma_start(out=t[:], in_=at_d[c * 128 : (c + 1) * 128, :])
                at_t.append(t)

            for b in range(BPC):
                # ---- load this batch's emb
                if b > 0:
                    for c in range(NCH):
                        t = epool.tile([128, S], F32R, tag="emb", name=f"emb{b}_{c}")
                        nc.sync.dma_start(out=t[:], in_=embT_d[b, c])
                        emb_all[b][c] = t
                emb_t = emb_all[b]
                lr_t = lr_all[b]

                # ---- stage 1: P = A_aug @ embaug.T  (db-outer over 8 banks);
                # the ones-row term (u) is folded into the copy bias below.
                st1 = [ps.tile([128, S], F32, tag="ps", name=f"st1_{b}_{da}")
                       for da in range(NCH)]
                prow_ps = ps.tile([1, S], F32, tag="ps")
                for db in range(NCH):
                    for da in range(NCH):
                        nc.tensor.matmul(st1[da][:],
                                         at_t[db][:, da * 128 : (da + 1) * 128],
                                         emb_t[db][:],
                                         start=(db == 0), stop=(db == NCH - 1))
                    # prow (P row 1024) rides the same chunk: 9 MMs per chunk
                    # pair matches the DMA arrival rate for batch 0
                    nc.tensor.matmul(prow_ps[:], at_t[db][:, D : D + 1],
                                     emb_t[db][:],
                                     start=(db == 0), stop=(db == NCH - 1))
                paug = []
                for da in range(NCH):
                    pt = paugpool.tile([128, S], F32R, tag="paug")
                    if da % 2 == 0:
                        nc.scalar.activation(out=pt[:], in_=st1[da][:],
                                             func=AFT.Identity,
                                             bias=ucol_t[:, da : da + 1], scale=1.0)
                    else:
                        nc.vector.tensor_scalar_add(pt[:], st1[da][:],
                                                    ucol_t[:, da : da + 1])
                    paug.append(pt)
                # P row 1024 (the bq-side rank-1 term); c0 folded into the bias
                prow = tiny.tile([1, S], F32R, tag="prow")
                nc.scalar.activation(out=prow[:], in_=prow_ps[:],
                                     func=AFT.Identity, bias=c0_t[:], scale=1.0)
                # rhs rows for the combined mask+prow matmul (K=3):
                # p0 = ones, p1 = m1neg (host), p2 = prow (device)
                rr3 = lrpool.tile([3, S], F32R, tag="rr3")
                nc.sync.dma_start(out=rr3[0:2, :], in_=rrows_d[b])
                nc.sync.dma_start(out=rr3[2:3, :], in_=prow[:])

                # ---- rsq / r / W2
                sqacc = sqpool.tile([128, S], F32R, tag="sqacc", bufs=2)
                sq0 = sqpool.tile([128, S], F32, tag="sq")
                nc.vector.tensor_mul(sq0[:], emb_t[0][:].bitcast(F32),
                                     emb_t[0][:].bitcast(F32))
                for c in range(1, NCH):
                    sq = sqpool.tile([128, S], F32, tag="sq")
                    nc.vector.tensor_mul(sq[:], emb_t[c][:].bitcast(F32),
                                         emb_t[c][:].bitcast(F32))
                    if c < NCH - 1:
                        nc.vector.tensor_add(sq0[:], sq0[:], sq[:])
                    else:
                        nc.vector.tensor_add(sqacc[:], sq0[:], sq[:])
                rsq_ps = ps.tile([1, S], F32, tag="ps")
                nc.tensor.matmul(rsq_ps[:], ones_col[:], sqacc[:],
                                 start=True, stop=True)
                s_row = tiny.tile([1, S], F32, tag="srow")
                nc.scalar.activation(out=s_row[:], in_=rsq_ps[:], func=AFT.Sqrt,
                                     bias=0.0, scale=1.0)
                r_row = tiny.tile([1, S], F32, tag="rrow")
                nc.vector.reciprocal(out=r_row[:], in_=s_row[:])
                nc.sync.dma_start(out=rout_d[b], in_=r_row[:])
                W2 = w2pool.tile([128, S], F32, tag="w2")
                nc.gpsimd.partition_broadcast(W2[:], r_row[0:1, :], channels=128)

                # ---- stage 2: L chunks + masks; per-chunk max
                mx = tiny.tile([128, NIC], F32, tag="mx")
                L_ps = []
                for ic in range(NIC):
                    Lp = ps.tile([128, S], F32, tag="ps", name=f"L_{b}_{ic}")
                    for da in range(NCH):
                        nc.tensor.matmul(Lp[:], emb_t[da][:, ic * 128 : (ic + 1) * 128],
                                         paug[da][:], start=(da == 0), stop=False)
                    nc.tensor.matmul(Lp[:], lr_t[:, ic * 128 : (ic + 1) * 128],
                                     rr3[:], start=False, stop=True)
                    nc.vector.reduce_max(mx[:, ic : ic + 1], Lp[:], axis=AX.X)
                    L_ps.append(Lp)

                # ---- global masked max -> -M in [128,1]
                par = tiny.tile([128, NIC], F32, tag="par")
                nc.gpsimd.partition_all_reduce(par[:], mx[:], channels=128,
                                               reduce_op=bass_isa.ReduceOp.max)
                negm128 = tiny.tile([128, 1], F32, tag="negm128")
                nc.vector.reduce_max(negm128[:], par[:], axis=AX.X, negate=True)

                # ---- gram chunks -> Gw = |G| * r_j
                gw_t = []
                for ic in range(NIC):
                    Gp = ps.tile([128, S], F32, tag="ps", name=f"G_{b}_{ic}")
                    for c in range(NCH):
                        nc.tensor.matmul(Gp[:], emb_t[c][:, ic * 128 : (ic + 1) * 128],
                                         emb_t[c][:], start=(c == 0), stop=(c == NCH - 1))
                    ga = gapool.tile([128, S], F32, tag="ga")
                    nc.scalar.activation(out=ga[:], in_=Gp[:], func=AFT.Abs,
                                         bias=0.0, scale=1.0)
                    gw = gwpool.tile([128, S], F32, tag="gw")
                    nc.vector.tensor_mul(gw[:], ga[:], W2[:])
                    gw_t.append(gw)

                # ---- exp + fused weighted reductions
                zwcols = tiny.tile([128, 2 * NIC], F32, tag="zwc")
                zcols = zwcols[:, 0:NIC]
                wcols = zwcols[:, NIC : 2 * NIC]
                for ic in range(NIC):
                    E = Epool.tile([128, S], F32, tag="E")
                    nc.scalar.activation(out=E[:], in_=L_ps[ic][:], func=AFT.Exp,
                                         bias=negm128[:], scale=1.0,
                                         accum_out=zcols[:, ic : ic + 1])
                    scr = scrpool.tile([128, S], F32, tag="scr")
                    nc.vector.scalar_tensor_tensor(
                        out=scr[:], in0=gw_t[ic][:], scalar=1.0, in1=E[:],
                        op0=ALU.mult, op1=ALU.mult,
                        accum_out=wcols[:, ic : ic + 1])

                nc.sync.dma_start(out=zw_d[b, 0], in_=zcols[:])
                nc.sync.dma_start(out=zw_d[b, 1], in_=wcols[:])

    nc.compile()
    _built = nc
    return nc


def kernel(embeddings, Wq, bq, Wk, bk, attention_masks, token_type_ids):
    global LAST_RESULTS
    nc = _build()

    embeddings = np.ascontiguousarray(np.asarray(embeddings, dtype=np.float32))
    Wq = np.asarray(Wq, dtype=np.float32)
    Wk = np.asarray(Wk, dtype=np.float32)
    bq = np.asarray(bq, dtype=np.float32)
    bk = np.asarray(bk, dtype=np.float32)
    am = np.asarray(attention_masks)
    tt = np.asarray(token_type_ids)

    # host-side layout + constant folding + fp32r rounding
    embT = _to_fp32r(embeddings.transpose(0, 2, 1)).reshape(B, NCH, 128, S)

    Wq64, Wk64 = Wq.astype(np.float64), Wk.astype(np.float64)
    A_aug = np.empty((DA, DA), np.float64)
    A_aug[:D, :D] = Wq64.T @ Wk64                  # A[d,d'] = sum_e Wq[e,d] Wk[e,d']
    A_aug[:D, D] = Wq64.T @ bk.astype(np.float64)   # u
    A_aug[D, :D] = Wk64.T @ bq.astype(np.float64)   # v
    A_aug[D, D] = float(bq.astype(np.float64) @ bk.astype(np.float64))
    AT = _to_fp32r(np.ascontiguousarray(A_aug.T).astype(np.float32))

    tok = am == 1
    m0 = tok & (tt == 0)
    m1 = tok & (tt == 1)
    m0neg = np.where(m0, np.float32(0.0), NEG).astype(np.float32)
    m1neg = np.where(m1, np.float32(0.0), NEG).astype(np.float32)
    ones_row = np.ones((B, 1, S), np.float32)
    lrows = _to_fp32r(np.concatenate([m0neg[:, None, :], ones_row, ones_row], axis=1))
    rrows = _to_fp32r(np.concatenate([ones_row, m1neg[:, None, :]], axis=1))
    ucol = np.ascontiguousarray(
        A_aug[:D, D].astype(np.float32).reshape(NCH, 128).T)        # [128, NCH]
    c0 = np.array([[A_aug[D, D]]], np.float32)

    in_maps = []
    for i in range(NCORES):
        sl = slice(i * BPC, (i + 1) * BPC)
        in_maps.append({
            "embT": np.ascontiguousarray(embT[sl]),
            "at": AT,
            "lrows": np.ascontiguousarray(lrows[sl]),
            "rrows": np.ascontiguousarray(rrows[sl]),
            "ones": np.ones((128, 1), np.float32),
            "onesrow": np.ones((1, S), np.float32),
            "ucol": ucol, "c0": c0,
        })

    res = run_bass_kernel_spmd(nc, in_maps, core_ids=list(range(NCORES)),
                               trace=PROFILE)
    LAST_RESULTS = res

    valid = m0.any(axis=1) & m1.any(axis=1)
    cs = np.zeros(B, np.float64)
    for i in range(NCORES):
        for j in range(BPC):
            b = i * BPC + j
            if not valid[b]:
                continue
            zcols = res.results[i]["zw"][j, 0].astype(np.float64)   # [128, NIC]
            wcols = res.results[i]["zw"][j, 1].astype(np.float64)
            r = res.results[i]["rout"][j].astype(np.float64)        # [S]
            ri = r.reshape(NIC, 128).T                              # [128, NIC]
            z = zcols.sum()
            w = (wcols * ri).sum()
            cs[b] = w / (z + 1e-30)
    return cs.astype(np.float32)



# revision 9
# speedup vs baseline: 2.4388x; 2.4388x over previous
"""Trainium2 Bass kernel for nn_CESAR_24309514895978 (ragged_sequence).

Math (per batch b):
  m0 = (attention_masks==1)&(token_type_ids==0); m1 = (attention_masks==1)&(token_type_ids==1)
  score[i,j] = |emb_n[i] . emb_n[j]|   (L2-normalized embeddings)
  logits[i,j] = (emb@Wq.T+bq)[i] . (emb@Wk.T+bk)[j]
  cs[b] = sum_{valid ij} softmax_flat(logits | pair_mask)[i,j] * score[i,j]

Key optimizations over the dense version:
  * Only i in m0 and j in m1 matter (the pair mask kills everything else), so
    the host gathers the ~130 valid tokens per side into SLOT-wide zero-padded
    slots.  The two batches of a core share one merged axis of 2*SLOT columns
    (batch 0 in cols [0,SLOT), batch 1 in [SLOT,2*SLOT)), with rank-4 mask rows
    handling cross-batch/pad exclusion.  This cuts every matmul's free dim from
    512 to 2*SLOT=320 and the i-chunks from 4 to 3.
  * All matmul inputs in bf16 (PE runs 1 cycle/row at any free size; fp32r
    would need free>=256 and 4 bytes of DMA per element).
  * The j-side gram operand is L2-normalized on the host, removing the on-chip
    sqrt/reciprocal/broadcast chain; with only Identity/Exp/Abs left, all
    activations live in one table set (exp_and_others) -> one ACT_TABLE_LOAD.
  * Constant folding: logits = e0aug @ A_aug @ e1aug.T with
    A_aug = [[Wq.T@Wk, Wq.T@bk], [bq.T@Wk, bq.bk]]; the u-column rides the
    PSUM->SBUF copy bias, prow (row 1024 of P) + masks ride one K=4 matmul.
  * DMA issues spread across sync/gpsimd/scalar sequencers (a single DGE
    config costs ~600ns serialized per engine).

Device per core: stage1 P = A_aug @ e1augT (8 db x (7+prow+1) = 72 mm),
stage2 L chunks + mask matmul (27 mm), gram (24 mm); exp/abs/stt with fused
per-partition accumulation -> zw [128, 2*IC].  Host: r_i scaling + final
W/Z division (tiny).
"""
import numpy as np
import ml_dtypes

import concourse.bass_isa as bass_isa
import concourse.tile as tile
from concourse import bacc, mybir
from concourse.bass_utils import run_bass_kernel_spmd

B, S, D = 16, 512, 1024
NCORES = 8
BPC = B // NCORES          # batches per core
NCH = D // 128             # 8 contraction chunks
DA = D + 1                 # augmented dim
NEG = np.float32(-1e30)
FILL = np.float32(-3e38)   # pre-fill for the per-chunk max tile

F32 = mybir.dt.float32
BF16 = mybir.dt.bfloat16
AFT = mybir.ActivationFunctionType
ALU = mybir.AluOpType
AX = mybir.AxisListType

PROFILE = False            # set True (e.g. from test.py) to capture NTFF profile
LAST_RESULTS = None        # BassKernelResults of the last run (for test.py)

_built = {}


def _ic_slices(ntot):
    return [(lo, min(lo + 128, ntot)) for lo in range(0, ntot, 128)]


def _build(slot):
    if slot in _built:
        return _built[slot]

    ntot = 2 * slot
    ics = _ic_slices(ntot)
    nic = len(ics)

    nc = bacc.Bacc("TRN2", target_bir_lowering=False, debug=False)

    at_d = nc.dram_tensor("at", [128, NCH * DA], BF16, kind="ExternalInput").ap()
    e1t_d = nc.dram_tensor("e1t", [128, NCH * ntot], BF16, kind="ExternalInput").ap()
    e0t_d = nc.dram_tensor("e0t", [128, NCH * ntot], BF16, kind="ExternalInput").ap()
    e1nt_d = nc.dram_tensor("e1nt", [128, NCH * ntot], BF16, kind="ExternalInput").ap()
    lr_d = nc.dram_tensor("lr", [4, ntot], BF16, kind="ExternalInput").ap()
    rr_d = nc.dram_tensor("rr", [3, ntot], BF16, kind="ExternalInput").ap()
    uc_d = nc.dram_tensor("uc", [128, 9 + nic], F32, kind="ExternalInput").ap()

    zw_d = nc.dram_tensor("zw", [128, 2 * nic], F32, kind="ExternalOutput").ap()

    with tile.TileContext(nc) as tc:
        with (
            tc.tile_pool(name="atp", bufs=1) as atp,
            tc.tile_pool(name="e1p", bufs=1) as e1p,
            tc.tile_pool(name="e0p", bufs=1) as e0p,
            tc.tile_pool(name="e1np", bufs=1) as e1np,
            tc.tile_pool(name="paugp", bufs=NCH) as paugp,
            tc.tile_pool(name="smallp", bufs=1) as smallp,
            tc.tile_pool(name="Ep", bufs=nic) as Ep,
            tc.tile_pool(name="gap", bufs=2) as gap,
            tc.tile_pool(name="scrp", bufs=2) as scrp,
            tc.tile_pool(name="ps", bufs=8, space="PSUM") as ps,
        ):
            # ---- DMA issues.  sync: at chunks (stage1 lhsT, critical path),
            # then e0t/e1nt.  gpsimd: e1t chunks (stage1 rhs) + smalls.
            at_t = atp.tile([128, NCH * DA], BF16, tag="at")
            e1t = e1p.tile([128, NCH * ntot], BF16, tag="e1")
            e0t = e0p.tile([128, NCH * ntot], BF16, tag="e0")
            e1nt = e1np.tile([128, NCH * ntot], BF16, tag="e1n")
            for c in range(NCH):
                nc.sync.dma_start(out=at_t[:, c * DA : (c + 1) * DA],
                                  in_=at_d[:, c * DA : (c + 1) * DA])
            for c in range(NCH):
                nc.gpsimd.dma_start(out=e1t[:, c * ntot : (c + 1) * ntot],
                                    in_=e1t_d[:, c * ntot : (c + 1) * ntot])
            nc.sync.dma_start(out=e0t[:], in_=e0t_d)
            nc.sync.dma_start(out=e1nt[:], in_=e1nt_d)
            lr_t = smallp.tile([4, ntot], BF16, tag="lr")
            nc.gpsimd.dma_start(out=lr_t[:], in_=lr_d)
            # rr row 0 = prow (ACT-written, must start at partition 0);
            # rows 1..3 = host mask rows R1/R2/NEGrow
            rr_t = smallp.tile([4, ntot], BF16, tag="rr")
            nc.gpsimd.dma_start(out=rr_t[1:4, :], in_=rr_d)
            uc_t = smallp.tile([128, 9 + nic], F32, tag="uc")
            nc.gpsimd.dma_start(out=uc_t[:], in_=uc_d)

            def atsl(db, da_lo, da_hi):
                return at_t[:, db * DA + da_lo : db * DA + da_hi]

            def e1sl(db):
                return e1t[:, db * ntot : (db + 1) * ntot]

            # ---- stage 1: P = A_aug @ e1augT.  db-outer over PSUM banks.
            # pass 1: das 0..6 + prow (8 banks); pass 2: da 7 reuses prow's
            # bank after the tiny prow copy.
            prow_ps = ps.tile([1, ntot], F32, tag="ps", name="prow_ps")
            P_ps = [
                ps.tile([128, ntot], F32, tag="ps", name=f"P{da}") for da in range(7)
            ]
            for db in range(NCH):
                st = db == 0
                sp = db == NCH - 1
                for da in range(7):
                    nc.tensor.matmul(
                        P_ps[da][:], atsl(db, da * 128, (da + 1) * 128), e1sl(db),
                        start=st, stop=sp,
                    )
                nc.tensor.matmul(
                    prow_ps[:], atsl(db, D, DA), e1sl(db), start=st, stop=sp
                )
            # prow -> rr row 0, with +c0 bias (rank-1 bq-side term)
            nc.scalar.activation(out=rr_t[0:1, :], in_=prow_ps[:], func=AFT.Identity,
                                 bias=uc_t[0:1, 8:9], scale=1.0)
            P7 = ps.tile([128, ntot], F32, tag="ps", name="P7")
            for db in range(NCH):
                nc.tensor.matmul(
                    P7[:], atsl(db, 7 * 128, D), e1sl(db),
                    start=(db == 0), stop=(db == NCH - 1),
                )
            P_ps.append(P7)

            # ---- PSUM->SBUF copies with the u-column bias, ACT/DVE split
            paug = []
            for da in range(NCH):
                pt = paugp.tile([128, ntot], BF16, tag="paug", name=f"paug{da}")
                if da % 2 == 0:
                    nc.scalar.activation(out=pt[:], in_=P_ps[da][:],
                                         func=AFT.Identity,
                                         bias=uc_t[:, da : da + 1], scale=1.0)
                else:
                    nc.vector.tensor_scalar_add(pt[:], P_ps[da][:],
                                                uc_t[:, da : da + 1])
                paug.append(pt)

            # ---- stage 2: L chunks + rank-4 mask/prow matmul; per-chunk max
            mx = uc_t[:, 9 : 9 + nic]  # pre-filled with -3e38 from host
            L_ps = []
            for ic, (lo, hi) in enumerate(ics):
                m = hi - lo
                Lp = ps.tile([128, ntot], F32, tag="ps", name=f"L{ic}")
                for da in range(NCH):
                    nc.tensor.matmul(
                        Lp[0:m, :], e0t[:, da * ntot + lo : da * ntot + hi],
                        paug[da][:], start=(da == 0), stop=False,
                    )
                nc.tensor.matmul(Lp[0:m, :], lr_t[:, lo:hi], rr_t[:],
                                 start=False, stop=True)
                nc.vector.reduce_max(mx[0:m, ic : ic + 1], Lp[0:m, :], axis=AX.X)
                L_ps.append(Lp)

            # ---- global masked max -> -M in [128,1]
            par = smallp.tile([128, nic], F32, tag="par")
            nc.gpsimd.partition_all_reduce(par[:], mx[:], channels=128,
                                           reduce_op=bass_isa.ReduceOp.max)
            negm = smallp.tile([128, 1], F32, tag="negm")
            nc.vector.reduce_max(negm[:], par[:], axis=AX.X, negate=True)

            # ---- gram chunks (j-side pre-normalized on host)
            G_ps = []
            for ic, (lo, hi) in enumerate(ics):
                m = hi - lo
                Gp = ps.tile([128, ntot], F32, tag="ps", name=f"G{ic}")
                for c in range(NCH):
                    nc.tensor.matmul(
                        Gp[0:m, :], e0t[:, c * ntot + lo : c * ntot + hi],
                        e1nt[:, c * ntot : (c + 1) * ntot],
                        start=(c == 0), stop=(c == NCH - 1),
                    )
                G_ps.append(Gp)

            # ---- exp + |G| + fused weighted reductions
            zw_t = smallp.tile([128, 2 * nic], F32, tag="zw")
            for ic, (lo, hi) in enumerate(ics):
                m = hi - lo
                E = Ep.tile([128, ntot], F32, tag="E", name=f"E{ic}")
                nc.scalar.activation(out=E[0:m, :], in_=L_ps[ic][0:m, :],
                                     func=AFT.Exp, bias=negm[0:m, :], scale=1.0,
                                     accum_out=zw_t[0:m, ic : ic + 1])
                ga = gap.tile([128, ntot], F32, tag="ga", name=f"ga{ic}")
                nc.scalar.activation(out=ga[0:m, :], in_=G_ps[ic][0:m, :],
                                     func=AFT.Abs, bias=0.0, scale=1.0)
                scr = scrp.tile([128, ntot], F32, tag="scr", name=f"scr{ic}")
                nc.vector.scalar_tensor_tensor(
                    out=scr[0:m, :], in0=ga[0:m, :], scalar=1.0, in1=E[0:m, :],
                    op0=ALU.mult, op1=ALU.mult,
                    accum_out=zw_t[0:m, nic + ic : nic + ic + 1])

            nc.sync.dma_start(out=zw_d, in_=zw_t[:])

    nc.compile()
    _built[slot] = nc
    return nc


def kernel(embeddings, Wq, bq, Wk, bk, attention_masks, token_type_ids):
    global LAST_RESULTS

    emb = np.ascontiguousarray(np.asarray(embeddings, dtype=np.float32))
    Wq = np.asarray(Wq, dtype=np.float32)
    Wk = np.asarray(Wk, dtype=np.float32)
    bq = np.asarray(bq, dtype=np.float32)
    bk = np.asarray(bk, dtype=np.float32)
    am = np.asarray(attention_masks)
    tt = np.asarray(token_type_ids)

    tok = am == 1
    m0 = tok & (tt == 0)
    m1 = tok & (tt == 1)
    n0 = m0.sum(1)
    n1 = m1.sum(1)

    slot = max(160, int(-(-max(n0.max(), n1.max()) // 32)) * 32)
    ntot = 2 * slot
    ics = _ic_slices(ntot)
    nic = len(ics)
    nc = _build(slot)

    # ---- constant folding (host, fp64 for accuracy)
    Wq64, Wk64 = Wq.astype(np.float64), Wk.astype(np.float64)
    A_aug = np.empty((DA, DA), np.float64)
    A_aug[:D, :D] = Wq64.T @ Wk64
    A_aug[:D, D] = Wq64.T @ bk.astype(np.float64)    # u
    A_aug[D, :D] = Wk64.T @ bq.astype(np.float64)    # v
    A_aug[D, D] = float(bq.astype(np.float64) @ bk.astype(np.float64))
    # at[p, db, da] = A_aug[da, db*128+p]
    at = np.ascontiguousarray(
        A_aug.T[:D].astype(np.float32).reshape(NCH, 128, DA).transpose(1, 0, 2)
    ).astype(ml_dtypes.bfloat16).reshape(128, NCH * DA)

    uc = np.zeros((128, 9 + nic), np.float32)
    uc[:, :NCH] = A_aug[:D, D].astype(np.float32).reshape(NCH, 128).T
    uc[0, 8] = A_aug[D, D]
    uc[:, 9:] = FILL

    # ---- per-batch gather into SLOT-wide zero-padded slots
    e0pack = np.zeros((B, slot, D), np.float32)
    e1pack = np.zeros((B, slot, D), np.float32)
    e1npack = np.zeros((B, slot, D), np.float32)
    r0 = np.zeros((B, slot), np.float64)
    for b in range(B):
        g0 = emb[b, m0[b]]
        g1 = emb[b, m1[b]]
        e0pack[b, : n0[b]] = g0
        e1pack[b, : n1[b]] = g1
        nrm1 = np.linalg.norm(g1.astype(np.float64), axis=1, keepdims=True)
        e1npack[b, : n1[b]] = g1 / np.maximum(nrm1, 1e-12)
        nrm0 = np.linalg.norm(g0.astype(np.float64), axis=1)
        r0[b, : n0[b]] = 1.0 / np.maximum(nrm0, 1e-12)

    def to_chunks(x2):  # [ntot, D] -> [128, NCH*ntot] bf16
        return np.ascontiguousarray(
            x2.T.reshape(NCH, 128, ntot).transpose(1, 0, 2)
        ).astype(ml_dtypes.bfloat16).reshape(128, NCH * ntot)

    in_maps = []
    for i in range(NCORES):
        b0, b1 = BPC * i, BPC * i + 1
        e0all = np.concatenate([e0pack[b0], e0pack[b1]], 0)
        e1all = np.concatenate([e1pack[b0], e1pack[b1]], 0)
        e1nall = np.concatenate([e1npack[b0], e1npack[b1]], 0)

        # lr rows pair with rr rows [prow(dev), R1, R2, NEGrow]
        lr = np.zeros((4, ntot), np.float32)
        lr[0] = 1.0                          # ones x prow
        lr[1, : n0[b0]] = 1.0                # A1 x R1
        lr[2, slot : slot + n0[b1]] = 1.0    # A2 x R2
        lr[3] = 1.0 - lr[1] - lr[2]          # Apad x NEGrow
        rr = np.full((3, ntot), NEG, np.float32)
        rr[0, : n1[b0]] = 0.0
        rr[1, slot : slot + n1[b1]] = 0.0

        in_maps.append({
            "at": at,
            "e1t": to_chunks(e1all),
            "e0t": to_chunks(e0all),
            "e1nt": to_chunks(e1nall),
            "lr": lr.astype(ml_dtypes.bfloat16),
            "rr": rr.astype(ml_dtypes.bfloat16),
            "uc": uc,
        })

    res = run_bass_kernel_spmd(nc, in_maps, core_ids=list(range(NCORES)),
                               trace=PROFILE)
    LAST_RESULTS = res

    # ---- host reduction: map merged-axis rows back to batches, apply r_i
    valid = m0.any(axis=1) & m1.any(axis=1)
    cs = np.zeros(B, np.float64)
    for i in range(NCORES):
        zw = res.results[i]["zw"].astype(np.float64)  # [128, 2*nic]
        for s in range(BPC):
            b = BPC * i + s
            if not valid[b]:
                continue
            g = slot * s + np.arange(n0[b])        # merged-axis rows
            ics_idx = g // 128
            p_idx = g % 128
            z = zw[p_idx, ics_idx].sum()
            w = (zw[p_idx, nic + ics_idx] * r0[b, : n0[b]]).sum()
            cs[b] = w / (z + 1e-300)
    return cs.astype(np.float32)


# revision 13
# speedup vs baseline: 2.6143x; 1.0720x over previous
"""Trainium2 Bass kernel for nn_CESAR_24309514895978 (ragged_sequence).

Math (per batch b):
  m0 = (attention_masks==1)&(token_type_ids==0); m1 = (attention_masks==1)&(token_type_ids==1)
  score[i,j] = |emb_n[i] . emb_n[j]|   (L2-normalized embeddings)
  logits[i,j] = (emb@Wq.T+bq)[i] . (emb@Wk.T+bk)[j]
  cs[b] = sum_{valid ij} softmax_flat(logits | pair_mask)[i,j] * score[i,j]

Key optimizations over the dense version:
  * Only i in m0 and j in m1 matter (the pair mask kills everything else).
    The host gathers valid tokens per side; the two batches of a core are
    packed CONTIGUOUSLY on one merged axis (batch 1 starts at the runtime
    boundary n_b0), padded to a compile-time ntot (~272 vs dense 512).
    Rank-4 mask rows handle cross-batch/pad exclusion.
  * All matmul inputs bf16: PE runs 1 cycle/row at any free size, LDWEIGHTS
    halves vs fp32r, DMA bytes halve.  rel-err stays ~1e-2 < 2e-2 because
    the flattened softmax is peaked and bf16 logit noise largely cancels
    between numerator and denominator.
  * j-side gram operand L2-normalized on host (no on-chip sqrt/recip chain);
    only Identity/Exp/Copy remain -> one ACT_TABLE_LOAD (exp_and_others).
  * Per-PARTITION (row) softmax max, shipped to host with Z/W partials: no
    cross-chunk all-reduce on device, exp fires right after each L chunk,
    and the host rescales with exp(M_row - M_batch) in fp64.
  * Constant folding: logits = e0aug @ A_aug @ e1aug.T,
    A_aug = [[Wq.T@Wk, Wq.T@bk], [bq.T@Wk, bq.bk]]; u-column rides the
    PSUM->SBUF copy bias, prow+masks ride one K=4 matmul.
  * PE warm-up matmuls on a zeroed tile during the DMA lead-in (TRN2 p-state
    ramps to 2.4GHz only after ~3us of continuous PE activity).
  * Stage-2 is da-outer so each P chunk is consumed right as its PSUM->SBUF
    copy lands; stage-1 is db-outer to match the at-chunk DMA arrival order.
  * DMA issues spread across sync/scalar/gpsimd sequencers (a DGE config
    costs ~700ns serialized per engine) with a tiny first at-slice so the
    first matmul's weights land early.
"""
import numpy as np
import ml_dtypes

import concourse.tile as tile
from concourse import bacc, mybir
from concourse.bass_utils import run_bass_kernel_spmd

B, S, D = 16, 512, 1024
NCORES = 8
BPC = B // NCORES          # batches per core
NCH = D // 128             # 8 contraction chunks
DA = D + 1                 # augmented dim
NEG = np.float32(-1e30)

F32 = mybir.dt.float32
BF16 = mybir.dt.bfloat16
AFT = mybir.ActivationFunctionType
ALU = mybir.AluOpType
AX = mybir.AxisListType

PROFILE = False            # set True (e.g. from test.py) to capture NTFF profile
LAST_RESULTS = None        # BassKernelResults of the last run (for test.py)

_built = {}


def _ic_slices(ntot):
    return [(lo, min(lo + 128, ntot)) for lo in range(0, ntot, 128)]


def _build(ntot):
    if ntot in _built:
        return _built[ntot]

    ics = _ic_slices(ntot)
    nic = len(ics)

    nc = bacc.Bacc("TRN2", target_bir_lowering=False, debug=False)

    at_d = nc.dram_tensor("at", [128, NCH * DA], BF16, kind="ExternalInput").ap()
    e1t_d = nc.dram_tensor("e1t", [128, NCH * ntot], BF16, kind="ExternalInput").ap()
    e0t_d = nc.dram_tensor("e0t", [128, NCH * ntot], BF16, kind="ExternalInput").ap()
    e1nt_d = nc.dram_tensor("e1nt", [128, NCH * ntot], BF16, kind="ExternalInput").ap()
    lr_d = nc.dram_tensor("lr", [4, ntot], BF16, kind="ExternalInput").ap()
    rr_d = nc.dram_tensor("rr", [3, ntot], BF16, kind="ExternalInput").ap()
    uc_d = nc.dram_tensor("uc", [128, 9], F32, kind="ExternalInput").ap()

    # cols [0:nic]=Z partials, [nic:2nic]=W partials, [2nic:3nic]=-rowmax
    zw_d = nc.dram_tensor("zw", [128, 3 * nic], F32, kind="ExternalOutput").ap()

    with tile.TileContext(nc) as tc:
        with (
            tc.tile_pool(name="atp", bufs=1) as atp,
            tc.tile_pool(name="e1p", bufs=1) as e1p,
            tc.tile_pool(name="e0p", bufs=1) as e0p,
            tc.tile_pool(name="e1np", bufs=1) as e1np,
            tc.tile_pool(name="paugp", bufs=NCH) as paugp,
            tc.tile_pool(name="smallp", bufs=1) as smallp,
            tc.tile_pool(name="warmp", bufs=1) as warmp,
            tc.tile_pool(name="Ep", bufs=3) as Ep,
            tc.tile_pool(name="gap", bufs=2) as gap,
            tc.tile_pool(name="scrp", bufs=2) as scrp,
            tc.tile_pool(name="ps", bufs=8, space="PSUM") as ps,
        ):
            at_t = atp.tile([128, NCH * DA], BF16, tag="at")
            e1t = e1p.tile([128, NCH * ntot], BF16, tag="e1")
            e0t = e0p.tile([128, NCH * ntot], BF16, tag="e0")
            e1nt = e1np.tile([128, NCH * ntot], BF16, tag="e1n")

            # ---- PE warm-up: zeroed tile, no DMA deps -> PE busy from the
            # end of the preamble, p-state ramped before real data lands.
            warm = warmp.tile([128, 512], BF16, tag="warm")
            nc.scalar.memzero(warm[:])
            warm_ps = ps.tile([128, 512], F32, tag="ps", name="warm_ps")
            for _ in range(6):
                nc.tensor.matmul(warm_ps[:], warm[:, 0:128], warm[:],
                                 start=True, stop=True)

            # ---- DMA issues.  sync: at00 (first matmul's weights, tiny),
            # at0-rest, at even chunks, e0t, e1nt.  scalar: at odd chunks.
            # gpsimd: e1t chunks + smalls.
            nc.sync.dma_start(out=at_t[:, 0:128], in_=at_d[:, 0:128])
            nc.sync.dma_start(out=at_t[:, 128:DA], in_=at_d[:, 128:DA])
            for c in (2, 4, 6):
                nc.sync.dma_start(out=at_t[:, c * DA : (c + 1) * DA],
                                  in_=at_d[:, c * DA : (c + 1) * DA])
            for c in (1, 3, 5, 7):
                nc.scalar.dma_start(out=at_t[:, c * DA : (c + 1) * DA],
                                    in_=at_d[:, c * DA : (c + 1) * DA])
            nc.sync.dma_start(out=e0t[:], in_=e0t_d)
            nc.sync.dma_start(out=e1nt[:], in_=e1nt_d)
            for c in range(NCH):
                nc.gpsimd.dma_start(out=e1t[:, c * ntot : (c + 1) * ntot],
                                    in_=e1t_d[:, c * ntot : (c + 1) * ntot])
            lr_t = smallp.tile([4, ntot], BF16, tag="lr")
            nc.gpsimd.dma_start(out=lr_t[:], in_=lr_d)
            # rr row 0 = prow (ACT-written, must start at partition 0);
            # rows 1..3 = host mask rows R1/R2/NEGrow
            rr_t = smallp.tile([4, ntot], BF16, tag="rr")
            nc.gpsimd.dma_start(out=rr_t[1:4, :], in_=rr_d)
            uc_t = smallp.tile([128, 9], F32, tag="uc")
            nc.gpsimd.dma_start(out=uc_t[:], in_=uc_d)

            def atsl(db, lo, hi):
                return at_t[:, db * DA + lo : db * DA + hi]

            def e1sl(db):
                return e1t[:, db * ntot : (db + 1) * ntot]

            # ---- stage 1: P = A_aug @ e1augT, db-outer (DMA arrival order).
            # pass 1: das 0..6 + prow (8 banks); pass 2: da 7 reuses prow's
            # bank right after the tiny prow copy.
            prow_ps = ps.tile([1, ntot], F32, tag="ps", name="prow_ps")
            P_ps = [
                ps.tile([128, ntot], F32, tag="ps", name=f"P{da}") for da in range(7)
            ]
            for db in range(NCH):
                st = db == 0
                sp = db == NCH - 1
                for da in range(7):
                    nc.tensor.matmul(
                        P_ps[da][:], atsl(db, da * 128, (da + 1) * 128), e1sl(db),
                        start=st, stop=sp,
                    )
                nc.tensor.matmul(
                    prow_ps[:], atsl(db, D, DA), e1sl(db), start=st, stop=sp
                )
            # prow -> rr row 0, with +c0 bias (rank-1 bq-side term)
            nc.scalar.activation(out=rr_t[0:1, :], in_=prow_ps[:], func=AFT.Identity,
                                 bias=uc_t[0:1, 8:9], scale=1.0)
            P7 = ps.tile([128, ntot], F32, tag="ps", name="P7")
            for db in range(NCH):
                nc.tensor.matmul(
                    P7[:], atsl(db, 7 * 128, D), e1sl(db),
                    start=(db == 0), stop=(db == NCH - 1),
                )
            P_ps.append(P7)

            # ---- PSUM->SBUF copies with the u-column bias, ACT/DVE split
            paug = []
            for da in range(NCH):
                pt = paugp.tile([128, ntot], BF16, tag="paug", name=f"paug{da}")
                if da % 2 == 0:
                    nc.scalar.activation(out=pt[:], in_=P_ps[da][:],
                                         func=AFT.Identity,
                                         bias=uc_t[:, da : da + 1], scale=1.0)
                else:
                    nc.vector.tensor_scalar_add(pt[:], P_ps[da][:],
                                                uc_t[:, da : da + 1])
                paug.append(pt)

            # ---- stage 2: L chunks, da-outer (consumes paug as produced),
            # then the rank-4 mask/prow matmul and per-ROW max per chunk.
            L_ps = [
                ps.tile([128, ntot], F32, tag="ps", name=f"L{ic}")
                for ic in range(nic)
            ]
            zw_t = smallp.tile([128, 3 * nic], F32, tag="zw")
            for da in range(NCH):
                for ic, (lo, hi) in enumerate(ics):
                    m = hi - lo
                    nc.tensor.matmul(
                        L_ps[ic][0:m, :], e0t[:, da * ntot + lo : da * ntot + hi],
                        paug[da][:], start=(da == 0), stop=False,
                    )
            for ic, (lo, hi) in enumerate(ics):
                m = hi - lo
                nc.tensor.matmul(L_ps[ic][0:m, :], lr_t[:, lo:hi], rr_t[:],
                                 start=False, stop=True)
                nc.vector.reduce_max(zw_t[0:m, 2 * nic + ic : 2 * nic + ic + 1],
                                     L_ps[ic][0:m, :], axis=AX.X, negate=True)

            # ---- gram chunks (j-side pre-normalized on host), ic-outer so
            # each G finishes early for the abs/stt pipeline
            G_ps = []
            for ic, (lo, hi) in enumerate(ics):
                m = hi - lo
                Gp = ps.tile([128, ntot], F32, tag="ps", name=f"G{ic}")
                for c in range(NCH):
                    nc.tensor.matmul(
                        Gp[0:m, :], e0t[:, c * ntot + lo : c * ntot + hi],
                        e1nt[:, c * ntot : (c + 1) * ntot],
                        start=(c == 0), stop=(c == NCH - 1),
                    )
                G_ps.append(Gp)

            # ---- E = exp(L - rowmax) + Z row-accum on ACT (fires during
            # gram); W = sum (|G| * E) in ONE fused DVE stt per chunk:
            # (G abs_max 0) mult E, with row accumulation
            E_t = []
            for ic, (lo, hi) in enumerate(ics):
                m = hi - lo
                E = Ep.tile([128, ntot], BF16, tag="E", name=f"E{ic}")
                nc.scalar.activation(out=E[0:m, :], in_=L_ps[ic][0:m, :],
                                     func=AFT.Exp,
                                     bias=zw_t[0:m, 2 * nic + ic : 2 * nic + ic + 1],
                                     scale=1.0,
                                     accum_out=zw_t[0:m, ic : ic + 1])
                E_t.append(E)
            for ic, (lo, hi) in enumerate(ics):
                m = hi - lo
                ga = gap.tile([128, ntot], BF16, tag="ga", name=f"ga{ic}")
                nc.scalar.activation(out=ga[0:m, :], in_=G_ps[ic][0:m, :],
                                     func=AFT.Abs, bias=0.0, scale=1.0)
                scr = scrp.tile([128, ntot], BF16, tag="scr", name=f"scr{ic}")
                nc.vector.scalar_tensor_tensor(
                    out=scr[0:m, :], in0=ga[0:m, :], scalar=1.0,
                    in1=E_t[ic][0:m, :], op0=ALU.mult, op1=ALU.mult,
                    accum_out=zw_t[0:m, nic + ic : nic + ic + 1])

            nc.sync.dma_start(out=zw_d, in_=zw_t[:])

    nc.compile()
    _built[ntot] = nc
    return nc


def kernel(embeddings, Wq, bq, Wk, bk, attention_masks, token_type_ids):
    global LAST_RESULTS

    emb = np.ascontiguousarray(np.asarray(embeddings, dtype=np.float32))
    Wq = np.asarray(Wq, dtype=np.float32)
    Wk = np.asarray(Wk, dtype=np.float32)
    bq = np.asarray(bq, dtype=np.float32)
    bk = np.asarray(bk, dtype=np.float32)
    am = np.asarray(attention_masks)
    tt = np.asarray(token_type_ids)

    tok = am == 1
    m0 = tok & (tt == 0)
    m1 = tok & (tt == 1)
    n0 = m0.sum(1)
    n1 = m1.sum(1)

    # merged-axis width: max per-core pair sum, rounded up to 16
    pair0 = n0.reshape(NCORES, BPC).sum(1)
    pair1 = n1.reshape(NCORES, BPC).sum(1)
    ntot = int(-(-max(pair0.max(), pair1.max()) // 16)) * 16
    ics = _ic_slices(ntot)
    nic = len(ics)
    nc = _build(ntot)

    # ---- constant folding (host, fp64 for accuracy)
    Wq64, Wk64 = Wq.astype(np.float64), Wk.astype(np.float64)
    A_aug = np.empty((DA, DA), np.float64)
    A_aug[:D, :D] = Wq64.T @ Wk64
    A_aug[:D, D] = Wq64.T @ bk.astype(np.float64)    # u
    A_aug[D, :D] = Wk64.T @ bq.astype(np.float64)    # v
    A_aug[D, D] = float(bq.astype(np.float64) @ bk.astype(np.float64))
    # at[p, db*DA + da] = A_aug[da, db*128+p]
    at = np.ascontiguousarray(
        A_aug.T[:D].astype(np.float32).reshape(NCH, 128, DA).transpose(1, 0, 2)
    ).astype(ml_dtypes.bfloat16).reshape(128, NCH * DA)

    uc = np.zeros((128, 9), np.float32)
    uc[:, :NCH] = A_aug[:D, D].astype(np.float32).reshape(NCH, 128).T
    uc[0, 8] = A_aug[D, D]

    def to_chunks(x2):  # [ntot, D] -> [128, NCH*ntot] bf16
        return np.ascontiguousarray(
            x2.T.reshape(NCH, 128, ntot).transpose(1, 0, 2)
        ).astype(ml_dtypes.bfloat16).reshape(128, NCH * ntot)

    in_maps = []
    r0g = []     # per core: r_i of the merged i-axis rows
    for i in range(NCORES):
        b0, b1 = BPC * i, BPC * i + 1
        e0all = np.zeros((ntot, D), np.float32)
        e1all = np.zeros((ntot, D), np.float32)
        e1nall = np.zeros((ntot, D), np.float32)
        g00, g01 = emb[b0, m0[b0]], emb[b1, m0[b1]]
        g10, g11 = emb[b0, m1[b0]], emb[b1, m1[b1]]
        c0i, c1i = n0[b0], n1[b0]
        e0all[:c0i] = g00
        e0all[c0i : c0i + n0[b1]] = g01
        e1all[:c1i] = g10
        e1all[c1i : c1i + n1[b1]] = g11
        nr0 = np.linalg.norm(
            e0all[: c0i + n0[b1]].astype(np.float64), axis=1)
        r0g.append(1.0 / np.maximum(nr0, 1e-12))
        nr1 = np.linalg.norm(
            e1all[: c1i + n1[b1]].astype(np.float64), axis=1, keepdims=True)
        e1nall[: c1i + n1[b1]] = e1all[: c1i + n1[b1]] / np.maximum(nr1, 1e-12)

        # lr rows pair with rr rows [prow(dev), R1, R2, NEGrow]
        lr = np.zeros((4, ntot), np.float32)
        lr[0] = 1.0                              # ones x prow
        lr[1, :c0i] = 1.0                        # A1 x R1
        lr[2, c0i : c0i + n0[b1]] = 1.0          # A2 x R2
        lr[3] = 1.0 - lr[1] - lr[2]              # Apad x NEGrow
        rr = np.full((3, ntot), NEG, np.float32)
        rr[0, :c1i] = 0.0
        rr[1, c1i : c1i + n1[b1]] = 0.0

        in_maps.append({
            "at": at,
            "e1t": to_chunks(e1all),
            "e0t": to_chunks(e0all),
            "e1nt": to_chunks(e1nall),
            "lr": lr.astype(ml_dtypes.bfloat16),
            "rr": rr.astype(ml_dtypes.bfloat16),
            "uc": uc,
        })

    res = run_bass_kernel_spmd(nc, in_maps, core_ids=list(range(NCORES)),
                               trace=PROFILE)
    LAST_RESULTS = res

    # ---- host reduction: per-row partials -> per-batch softmax-weighted sum
    valid = m0.any(axis=1) & m1.any(axis=1)
    cs = np.zeros(B, np.float64)
    for i in range(NCORES):
        zw = res.results[i]["zw"].astype(np.float64)  # [128, 3*nic]
        b0 = BPC * i
        starts = [0, n0[b0]]
        for s in range(BPC):
            b = b0 + s
            if not valid[b]:
                continue
            g = starts[s] + np.arange(n0[b])      # merged-axis rows
            ic_idx = g // 128
            p_idx = g % 128
            zrow = zw[p_idx, ic_idx]
            wrow = zw[p_idx, nic + ic_idx]
            mrow = -zw[p_idx, 2 * nic + ic_idx]   # per-row max M_i
            mb = mrow.max()
            scale = np.exp(mrow - mb)
            z = (zrow * scale).sum()
            w = (wrow * scale * r0g[i][g]).sum()
            cs[b] = w / (z + 1e-300)
    return cs.astype(np.float32)
